# revision 13
# baseline (speedup 1.0000x reference)
"""GQA attention (S=2048, D=2048, 32 q-heads / 8 kv-heads, rope, causal) on 8
Trainium2 NeuronCores, tensor-parallel over heads (1 kv head + 4 q heads per
core), chunked AllToAll re-shard overlapped with compute, row-sharded output.

v2 layout/scheduling notes (on top of the v1 transposed-domain design):
 - rope is vectorized across a whole 128-partition psum tile: q heads are
   pair-packed ([h0 evens; h1 evens; h0 odds; h1 odds] rows) so one DVE mul
   against a host-built [cos;cos;-sin;-sin | sin;sin;cos;cos] table plus four
   32-row adds replaces 24 narrow ops per tile.
 - scores for the two heads of a pair run CONCURRENTLY on the PE via row
   tiling: kT is duplicated to partitions 64-127, q pairs live at [0:64] and
   [64:128], so the two K=64 matmuls occupy disjoint row groups.
 - softmax denominators: the PV stationary is [v | ones*64] so psum rows
   64-127 hold 64 replicated copies of the denominator -> reciprocal+scale are
   plain [64,512] DVE ops (no gpsimd partition_broadcast chain).
 - stage P (projections) and early attention tiles of head-pair 0 are
   interleaved so the scalar engine starts exp'ing while projections stream.
 - stage W runs in 2 psum banks with an SBUF accumulator so its first
   AllToAll chunk overlaps head-pair-1 attention; second chunk is the tail.
"""
import os
import sys
from contextlib import ExitStack

import numpy as np

try:
    import concourse.bass as bass  # noqa: F401
except ImportError:  # platform tree not on sys.path in a fresh dir
    sys.path.insert(0, "/opt/trn_rl_repo")
    import concourse.bass as bass  # noqa: F401

import concourse.mybir as mybir
from concourse import bacc, bass_utils, tile
from concourse.masks import make_identity

F32 = mybir.dt.float32
BF16 = mybir.dt.bfloat16
AF = mybir.ActivationFunctionType

S = 2048          # sequence length
D = 2048          # model dim
HD = 64           # head dim
N_CORES = 8
QCOLS = 256       # 4 q heads * 64 per core (2 pairs of 128)
KVCOLS = 128      # packed k(evens,odds)|v cols per core
ROWS_PER_CORE = S // N_CORES  # 256 output rows per core

# fraction of non-diagonal exp blocks computed on the vector engine via
# pow(e^0.125, x); 0 = all on scalar engine
DVE_EXP = False


def _build():
    nc = bacc.Bacc("TRN2", target_bir_lowering=False, debug=False,
                   num_devices=N_CORES)
    xT_d = nc.dram_tensor("xT", [2, 16, 128, 1024], BF16,
                          kind="ExternalInput")
    wq_d = nc.dram_tensor("wq", [128, 16, QCOLS], BF16, kind="ExternalInput")
    wkv_d = nc.dram_tensor("wkv", [128, 16, KVCOLS], BF16,
                           kind="ExternalInput")
    wo_d = nc.dram_tensor("wo", [128, 16, D], BF16, kind="ExternalInput")
    ropeM_d = nc.dram_tensor("ropeM", [128, 4, 1024], BF16,
                             kind="ExternalInput")
    ropeK_d = nc.dram_tensor("ropeK", [64, 4, 1024], BF16,
                             kind="ExternalInput")
    mask_d = nc.dram_tensor("maskT01", [128, 128], BF16, kind="ExternalInput")
    out_d = nc.dram_tensor("out", [ROWS_PER_CORE, D], F32,
                           kind="ExternalOutput")

    with tile.TileContext(nc) as tc, ExitStack() as top:
        persist = top.enter_context(tc.tile_pool(name="persist", bufs=1))
        qpairs = [persist.tile([128, S], BF16, name=f"qpair{p}",
                               uniquify=False) for p in range(2)]
        kdup = persist.tile([128, S], BF16, name="kdup")
        v2sb = persist.tile([128, 16, 128], BF16, name="v2sb")
        attnT0 = persist.tile([128, S], BF16, name="attnT0")
        attnT1 = persist.tile([128, S], BF16, name="attnT1")
        attnTs = [attnT0, attnT1]
        maskT_sb = persist.tile([128, 128], BF16, name="maskT_sb")
        nc.scalar.dma_start(maskT_sb[:], mask_d.ap())
        wo_sb = persist.tile([128, 16, D], BF16, name="wo_sb")
        if DVE_EXP:
            expbase = persist.tile([128, 512], F32, name="expbase")
            nc.vector.memset(expbase[:], float(np.exp(0.125)))

        dram = top.enter_context(tc.tile_pool(name="dram", bufs=1,
                                              space="DRAM"))
        a2a_in = [dram.tile([N_CORES, 128, ROWS_PER_CORE], BF16,
                            name=f"a2a_in{i}", uniquify=False)
                  for i in range(2)]
        a2a_out = [dram.tile([N_CORES, 128, ROWS_PER_CORE], BF16,
                             name=f"a2a_out{i}", uniquify=False)
                   for i in range(2)]

        probs_pool = top.enter_context(tc.tile_pool(name="probs", bufs=3))
        nrm_pool = top.enter_context(tc.tile_pool(name="nrm", bufs=1))

        def attn_group(p, t, psc_pool, po_pool):
            """Attention for head pair p (heads 2p, 2p+1), q-tile t."""
            qp = qpairs[p]
            nb = 4 * t + 4
            pos = [po_pool.tile([128, 512], F32, name=f"po{p}{t}{j}",
                                tag=f"po{j}") for j in range(2)]
            for b in range(nb):
                j = max(0, b - 4 * t)
                col0 = 128 * j
                diag = b >= 4 * t
                kb = kdup[:, 128 * b:128 * (b + 1)]
                qcols = (512 * t + col0, 512 * (t + 1))
                pscs = []
                for h in range(2):
                    psc = psc_pool.tile([128, 512], F32,
                                        name=f"ps{p}{t}{b}{h}", tag=f"psc{h}")
                    nc.tensor.matmul(
                        psc[:, col0:512], kb[64 * h:64 * (h + 1), :],
                        qp[64 * h:64 * (h + 1), qcols[0]:qcols[1]],
                        start=True, stop=True)
                    pscs.append(psc)
                prbs = []
                for h in range(2):
                    probs = probs_pool.tile([128, 512], BF16,
                                            name=f"pr{p}{t}{b}{h}",
                                            tag=f"probs{h}")
                    late = (p == 1) or (t == 3)
                    if DVE_EXP and late and (b + h) % 2 == 1:
                        nc.vector.tensor_tensor(
                            probs[:, col0:512], expbase[:, col0:512],
                            pscs[h][:, col0:512], mybir.AluOpType.pow)
                    else:
                        nc.scalar.activation(probs[:, col0:512],
                                             pscs[h][:, col0:512], AF.Exp,
                                             scale=0.125)
                    if diag:
                        nc.vector.tensor_mul(probs[:, col0:col0 + 128],
                                             probs[:, col0:col0 + 128],
                                             maskT_sb[:])
                    prbs.append(probs)
                for h in range(2):
                    nc.tensor.matmul(pos[h][:, col0:512], v2sb[:, b, :],
                                     prbs[h][:, col0:512],
                                     start=(b == 0), stop=(b == nb - 1))
            for h in range(2):
                # custom-DVE ops mis-handle a nonzero input base partition, so
                # stage the denominator rows at partition 0 first
                den = nrm_pool.tile([64, 512], F32, name=f"dn{p}{t}{h}",
                                    tag="den")
                nc.scalar.copy(den[:], pos[h][64:128, :])
                recip = nrm_pool.tile([64, 512], F32, name=f"rc{p}{t}{h}",
                                      tag="recip")
                nc.vector.reciprocal_approx_fast(recip[:], den[:])
                nc.vector.tensor_mul(
                    attnTs[p][64 * h:64 * (h + 1), 512 * t:512 * (t + 1)],
                    pos[h][0:64, :], recip[:])

        # ---------------- Stage P: projections + rope (+ early attn) -------
        with ExitStack() as ctx:
            wpool = ctx.enter_context(tc.tile_pool(name="wpool", bufs=1))
            wq_sb = wpool.tile([128, 16, QCOLS], BF16, name="wq_sb")
            wkv_sb = wpool.tile([128, 16, KVCOLS], BF16, name="wkv_sb")
            ropeM_sb = wpool.tile([128, 4, 1024], BF16, name="ropeM_sb")
            ropeK_sb = wpool.tile([64, 4, 1024], BF16, name="ropeK_sb")
            vsb = wpool.tile([64, S], F32, name="vsb")
            identity = wpool.tile([64, 64], F32, name="identity")
            make_identity(nc, identity[:])

            nc.gpsimd.dma_start(wkv_sb[:], wkv_d.ap())
            nc.gpsimd.dma_start(ropeM_sb[:], ropeM_d.ap())
            nc.gpsimd.dma_start(ropeK_sb[:], ropeK_d.ap())
            nc.vector.memset(v2sb[:, :, HD:], 1.0)

            xtb_pool = ctx.enter_context(tc.tile_pool(name="xtb", bufs=16))
            proj_pool = ctx.enter_context(
                tc.tile_pool(name="proj", bufs=1, space="PSUM"))
            pvt_pool = ctx.enter_context(
                tc.tile_pool(name="pvt", bufs=1, space="PSUM"))
            pscE_pool = ctx.enter_context(
                tc.tile_pool(name="pscE", bufs=1, space="PSUM"))
            poE_pool = ctx.enter_context(
                tc.tile_pool(name="poE", bufs=1, space="PSUM"))
            tmp_pool = ctx.enter_context(tc.tile_pool(name="ropetmp", bufs=1))

            xtbs = {}
            for sq in range(4):
                s0 = 512 * sq
                sh, so = sq // 2, 512 * (sq % 2)
                pq = [proj_pool.tile([128, 512], F32, name=f"pq{sq}_{p}",
                                     tag=f"pq{p}") for p in range(2)]
                pkv = proj_pool.tile([128, 512], F32, name=f"pkv{sq}",
                                     tag="pkv")
                for kc in range(16):
                    if sq == 0:
                        weng = nc.scalar if kc % 2 == 0 else nc.sync
                        weng.dma_start(wq_sb[:, kc, :], wq_d.ap()[:, kc, :])
                    if sq % 2 == 0:
                        xtb = xtb_pool.tile([128, 1024], BF16,
                                            name=f"xtb{sh}_{kc}", tag="xtb")
                        eng = nc.sync if kc % 2 == 0 else nc.scalar
                        eng.dma_start(xtb[:], xT_d.ap()[sh, kc])
                        xtbs[(sh, kc)] = xtb
                    xtb = xtbs[(sh, kc)]
                    st, sp = (kc == 0), (kc == 15)
                    for p in range(2):
                        nc.tensor.matmul(
                            pq[p][:], wq_sb[:, kc, 128 * p:128 * (p + 1)],
                            xtb[:, so:so + 512], start=st, stop=sp)
                    nc.tensor.matmul(pkv[:], wkv_sb[:, kc, :],
                                     xtb[:, so:so + 512], start=st, stop=sp)
                # rope q: per pair, 4 half-muls + 4 adds (lo/hi split keeps
                # every tensor_tensor's SBUF inputs at one start partition)
                for p in range(2):
                    ta = tmp_pool.tile([64, 1024], F32, name=f"ta{sq}{p}",
                                       tag="ta")
                    tb = tmp_pool.tile([64, 1024], F32, name=f"tb{sq}{p}",
                                       tag="tb")
                    nc.vector.tensor_mul(ta[:, 0:512], pq[p][0:64, :],
                                         ropeM_sb[0:64, sq, 0:512])
                    nc.vector.tensor_mul(ta[:, 512:1024], pq[p][0:64, :],
                                         ropeM_sb[0:64, sq, 512:1024])
                    nc.vector.tensor_mul(tb[:, 0:512], pq[p][64:128, :],
                                         ropeM_sb[64:128, sq, 0:512])
                    nc.vector.tensor_mul(tb[:, 512:1024], pq[p][64:128, :],
                                         ropeM_sb[64:128, sq, 512:1024])
                    qp = qpairs[p]
                    for h in range(2):
                        a0 = 32 * h
                        nc.vector.tensor_add(
                            qp[64 * h:64 * h + 32, s0:s0 + 512],
                            ta[a0:a0 + 32, 0:512], tb[a0:a0 + 32, 0:512])
                        nc.vector.tensor_add(
                            qp[64 * h + 32:64 * h + 64, s0:s0 + 512],
                            ta[a0:a0 + 32, 512:1024],
                            tb[a0:a0 + 32, 512:1024])
                # rope k (rows 0:64 of pkv) -> kdup rows 0:64, dma-dup to 64:128
                ka = tmp_pool.tile([32, 1024], F32, name=f"ka{sq}", tag="ka")
                kb = tmp_pool.tile([32, 1024], F32, name=f"kb{sq}", tag="kb")
                nc.vector.tensor_mul(ka[:, 0:512], pkv[0:32, :],
                                     ropeK_sb[0:32, sq, 0:512])
                nc.vector.tensor_mul(ka[:, 512:1024], pkv[0:32, :],
                                     ropeK_sb[0:32, sq, 512:1024])
                nc.vector.tensor_mul(kb[:, 0:512], pkv[32:64, :],
                                     ropeK_sb[32:64, sq, 0:512])
                nc.vector.tensor_mul(kb[:, 512:1024], pkv[32:64, :],
                                     ropeK_sb[32:64, sq, 512:1024])
                nc.vector.tensor_add(kdup[0:32, s0:s0 + 512],
                                     ka[:, 0:512], kb[:, 0:512])
                nc.vector.tensor_add(kdup[32:64, s0:s0 + 512],
                                     ka[:, 512:1024], kb[:, 512:1024])
                nc.sync.dma_start(kdup[64:128, s0:s0 + 512],
                                  kdup[0:64, s0:s0 + 512])
                # v -> vsb (f32) -> per-128-block transpose -> v2sb cols 0:64
                nc.scalar.copy(vsb[:, s0:s0 + 512], pkv[64:128, :])
                for sc in range(4 * sq, 4 * sq + 4):
                    pvt = pvt_pool.tile([128, 64], F32, name=f"pvt{sc}",
                                        tag="pvt")
                    nc.tensor.transpose(pvt[:], vsb[:, 128 * sc:128 * (sc + 1)],
                                        identity[:])
                    nc.scalar.copy(v2sb[:, sc, 0:HD], pvt[:])
                if sq >= 1:
                    attn_group(0, sq - 1, pscE_pool, poE_pool)
                if sq == 3:
                    attn_group(0, 3, pscE_pool, poE_pool)


        # ---------------- Phase 2: rest of attention + A2A + stage W --------
        with ExitStack() as ctx:
            pscL_pool = ctx.enter_context(
                tc.tile_pool(name="pscL", bufs=2, space="PSUM"))
            poL_pool = ctx.enter_context(
                tc.tile_pool(name="poL", bufs=1, space="PSUM"))
            pw_pool = ctx.enter_context(
                tc.tile_pool(name="pw", bufs=1, space="PSUM"))
            af_pool = ctx.enter_context(tc.tile_pool(name="af", bufs=1))
            osb_pool = ctx.enter_context(tc.tile_pool(name="osb", bufs=2))
            accp = ctx.enter_context(tc.tile_pool(name="accp", bufs=1))
            acc_sb = accp.tile([128, 8, 512], F32, name="acc_sb")

            def send_a2a(i):
                for r in range(N_CORES):
                    nc.sync.dma_start(a2a_in[i][r],
                                      attnTs[i][:, 256 * r:256 * (r + 1)])
                nc.gpsimd.collective_compute(
                    "AllToAll", mybir.AluOpType.bypass,
                    replica_groups=[list(range(N_CORES))],
                    ins=[a2a_in[i][:]], outs=[a2a_out[i][:]])

            afs = []

            def load_af(i):
                af = af_pool.tile([128, N_CORES, ROWS_PER_CORE], BF16,
                                  name=f"attn_full{i}", uniquify=False)
                nc.sync.dma_start(af[:],
                                  a2a_out[i][:].rearrange("r p s -> p r s"))
                afs.append(af)

            def w_subpass(i, m, n2):
                # accumulate out[128m:128m+128, 1024n2:1024n2+1024] over the 8
                # src cores of a2a chunk i, two psum banks (tags A/B)
                for nn in range(2):
                    n = 2 * n2 + nn
                    pw = pw_pool.tile([128, 512], F32, name=f"pw{i}{m}{n}",
                                      tag=f"pw{nn}")
                    for r in range(N_CORES):
                        nc.tensor.matmul(
                            pw[:], afs[i][:, r, 128 * m:128 * (m + 1)],
                            wo_sb[:, 2 * r + i, 512 * n:512 * (n + 1)],
                            start=(r == 0), stop=(r == N_CORES - 1))
                    if i == 0:
                        nc.vector.tensor_scalar_add(acc_sb[:, 4 * m + n, :],
                                                    pw[:], 0.0)
                    else:
                        osb = osb_pool.tile([128, 512], F32,
                                            name=f"osb{m}{n}", tag="osb")
                        nc.vector.tensor_add(osb[:], pw[:],
                                             acc_sb[:, 4 * m + n, :])
                        nc.sync.dma_start(
                            out_d.ap()[128 * m:128 * (m + 1),
                                       512 * n:512 * (n + 1)], osb[:])

            send_a2a(0)
            load_af(0)
            for kc in range(16):
                nc.gpsimd.tensor_scalar_add(wo_sb[:, kc, 0:1],
                                            kdup[:, 0:1], 0.0)
                nc.gpsimd.dma_start(wo_sb[:, kc, :], wo_d.ap()[:, kc, :])
            attn_group(1, 0, pscL_pool, poL_pool)
            attn_group(1, 1, pscL_pool, poL_pool)
            attn_group(1, 2, pscL_pool, poL_pool)
            attn_group(1, 3, pscL_pool, poL_pool)
            send_a2a(1)
            for m in range(2):
                for n2 in range(2):
                    w_subpass(0, m, n2)
            load_af(1)
            for m in range(2):
                for n2 in range(2):
                    w_subpass(1, m, n2)

    nc.compile()
    return nc


_NC_CACHE = None
LAST_RESULT = None


def _get_nc():
    global _NC_CACHE
    if _NC_CACHE is None:
        _NC_CACHE = _build()
    return _NC_CACHE


def kernel(x, wq, wk, wv, wo, freqs_cos, freqs_sin, mask, start_pos=0):
    assert int(start_pos) == 0, "kernel specialized for start_pos == 0"
    import ml_dtypes
    x = np.asarray(x, np.float32)
    b, s, d = x.shape
    assert (b, s, d) == (1, S, D)
    xT = np.ascontiguousarray(x[0].T).astype(ml_dtypes.bfloat16)
    # pre-tile: xT[sh, kc] = contiguous (128, 1024) block -> 2KB DMA lines
    xTt = np.ascontiguousarray(
        xT.reshape(16, 128, 2, 1024).transpose(2, 0, 1, 3))

    # wq pair-packed: per pair of heads, cols = [h0 evens, h1 evens,
    # h0 odds, h1 odds]
    wq_f = np.asarray(wq, np.float32).reshape(D, 32, 32, 2)
    wk_f = np.asarray(wk, np.float32).reshape(D, 8, 32, 2)
    wv_f = np.asarray(wv, np.float32)

    cosT = np.asarray(freqs_cos, np.float32).T  # (32, S)
    sinT = np.asarray(freqs_sin, np.float32).T
    ropeM = np.empty((128, 4, 1024), np.float32)
    ropeK = np.empty((64, 4, 1024), np.float32)
    for sq in range(4):
        c = cosT[:, 512 * sq:512 * (sq + 1)]
        sn = sinT[:, 512 * sq:512 * (sq + 1)]
        ropeM[0:32, sq, 0:512] = c
        ropeM[32:64, sq, 0:512] = c
        ropeM[64:96, sq, 0:512] = -sn
        ropeM[96:128, sq, 0:512] = -sn
        ropeM[0:32, sq, 512:] = sn
        ropeM[32:64, sq, 512:] = sn
        ropeM[64:96, sq, 512:] = c
        ropeM[96:128, sq, 512:] = c
        ropeK[0:32, sq, 0:512] = c
        ropeK[32:64, sq, 0:512] = -sn
        ropeK[0:32, sq, 512:] = sn
        ropeK[32:64, sq, 512:] = c
    ropeM_b = ropeM.astype(ml_dtypes.bfloat16)
    ropeK_b = ropeK.astype(ml_dtypes.bfloat16)

    wot = np.ascontiguousarray(
        np.asarray(wo, np.float32).reshape(16, 128, D).transpose(1, 0, 2)
    ).astype(ml_dtypes.bfloat16)
    maskT01 = np.ascontiguousarray(
        (np.asarray(mask, np.float32)[:128, :128].T == 0.0)
    ).astype(ml_dtypes.bfloat16)

    in_maps = []
    for c in range(N_CORES):
        # pair p cols: h=4c+2p, h2=4c+2p+1
        wq_cols = []
        for p in range(2):
            h0, h1 = 4 * c + 2 * p, 4 * c + 2 * p + 1
            wq_cols.append(wq_f[:, h0, :, 0])  # evens (D, 32)
            wq_cols.append(wq_f[:, h1, :, 0])
            wq_cols.append(wq_f[:, h0, :, 1])  # odds
            wq_cols.append(wq_f[:, h1, :, 1])
        wq_c = np.concatenate(wq_cols, axis=1)  # (D, 256)
        wkv_c = np.concatenate(
            [wk_f[:, c, :, 0], wk_f[:, c, :, 1],
             wv_f[:, HD * c:HD * (c + 1)]], axis=1)  # (D, 128)
        in_maps.append({
            "xT": xTt,
            "wq": np.ascontiguousarray(
                wq_c.reshape(16, 128, QCOLS).transpose(1, 0, 2)
            ).astype(ml_dtypes.bfloat16),
            "wkv": np.ascontiguousarray(
                wkv_c.reshape(16, 128, KVCOLS).transpose(1, 0, 2)
            ).astype(ml_dtypes.bfloat16),
            "wo": wot,
            "ropeM": ropeM_b,
            "ropeK": ropeK_b,
            "maskT01": maskT01,
        })

    nc = _get_nc()
    res = bass_utils.run_bass_kernel_spmd(
        nc, in_maps, core_ids=list(range(N_CORES)),
        trace=bool(os.environ.get("BASS_TRACE")))
    global LAST_RESULT
    LAST_RESULT = res
    rows = [res.results[c]["out"] for c in range(N_CORES)]
    return np.concatenate(rows, axis=0).reshape(1, S, D).astype(np.float32)


# revision 14
# speedup vs baseline: 1.0678x; 1.0678x over previous
"""GQA attention (S=2048, D=2048, 32 q-heads / 8 kv-heads, rope, causal) on 8
Trainium2 NeuronCores, tensor-parallel over heads (1 kv head + 4 q heads per
core), chunked AllToAll re-shard overlapped with compute, row-sharded output.

v2 layout/scheduling notes (on top of the v1 transposed-domain design):
 - rope is vectorized across a whole 128-partition psum tile: q heads are
   pair-packed ([h0 evens; h1 evens; h0 odds; h1 odds] rows) so one DVE mul
   against a host-built [cos;cos;-sin;-sin | sin;sin;cos;cos] table plus four
   32-row adds replaces 24 narrow ops per tile.
 - scores for the two heads of a pair run CONCURRENTLY on the PE via row
   tiling: kT is duplicated to partitions 64-127, q pairs live at [0:64] and
   [64:128], so the two K=64 matmuls occupy disjoint row groups.
 - softmax denominators: the PV stationary is [v | ones*64] so psum rows
   64-127 hold 64 replicated copies of the denominator -> reciprocal+scale are
   plain [64,512] DVE ops (no gpsimd partition_broadcast chain).
 - stage P (projections) and early attention tiles of head-pair 0 are
   interleaved so the scalar engine starts exp'ing while projections stream.
 - stage W runs in 2 psum banks with an SBUF accumulator so its first
   AllToAll chunk overlaps head-pair-1 attention; second chunk is the tail.
"""
import os
import sys
from contextlib import ExitStack

import numpy as np

try:
    import concourse.bass as bass  # noqa: F401
except ImportError:  # platform tree not on sys.path in a fresh dir
    sys.path.insert(0, "/opt/trn_rl_repo")
    import concourse.bass as bass  # noqa: F401

import concourse.mybir as mybir
from concourse import bacc, bass_utils, tile
from concourse.masks import make_identity

F32 = mybir.dt.float32
BF16 = mybir.dt.bfloat16
AF = mybir.ActivationFunctionType

S = 2048          # sequence length
D = 2048          # model dim
HD = 64           # head dim
N_CORES = 8
QCOLS = 256       # 4 q heads * 64 per core (2 pairs of 128)
KVCOLS = 128      # packed k(evens,odds)|v cols per core
ROWS_PER_CORE = S // N_CORES  # 256 output rows per core

# fraction of non-diagonal exp blocks computed on the vector engine via
# pow(e^0.125, x); 0 = all on scalar engine
DVE_EXP = False


def _build():
    nc = bacc.Bacc("TRN2", target_bir_lowering=False, debug=False,
                   num_devices=N_CORES)
    xT_d = nc.dram_tensor("xT", [2, 16, 128, 1024], BF16,
                          kind="ExternalInput")
    wq_d = nc.dram_tensor("wq", [128, 16, QCOLS], BF16, kind="ExternalInput")
    wkv_d = nc.dram_tensor("wkv", [128, 16, KVCOLS], BF16,
                           kind="ExternalInput")
    wo_d = nc.dram_tensor("wo", [128, 16, D], BF16, kind="ExternalInput")
    ropeM_d = nc.dram_tensor("ropeM", [128, 4, 1024], BF16,
                             kind="ExternalInput")
    ropeK_d = nc.dram_tensor("ropeK", [64, 4, 1024], BF16,
                             kind="ExternalInput")
    mask_d = nc.dram_tensor("maskT01", [128, 128], BF16, kind="ExternalInput")
    out_d = nc.dram_tensor("out", [ROWS_PER_CORE, D], F32,
                           kind="ExternalOutput")

    with tile.TileContext(nc) as tc, ExitStack() as top:
        persist = top.enter_context(tc.tile_pool(name="persist", bufs=1))
        qpairs = [persist.tile([128, S], BF16, name=f"qpair{p}",
                               uniquify=False) for p in range(2)]
        kdup = persist.tile([128, S], BF16, name="kdup")
        v2sb = persist.tile([128, 16, 128], BF16, name="v2sb")
        attnT0 = persist.tile([128, S], BF16, name="attnT0")
        attnT1 = persist.tile([128, S], BF16, name="attnT1")
        attnTs = [attnT0, attnT1]
        maskT_sb = persist.tile([128, 128], BF16, name="maskT_sb")
        nc.scalar.dma_start(maskT_sb[:], mask_d.ap())
        wo_sb = persist.tile([128, 16, D], BF16, name="wo_sb")
        if DVE_EXP:
            expbase = persist.tile([128, 512], F32, name="expbase")
            nc.vector.memset(expbase[:], float(np.exp(0.125)))

        dram = top.enter_context(tc.tile_pool(name="dram", bufs=1,
                                              space="DRAM"))
        a2a_in = [dram.tile([N_CORES, 128, ROWS_PER_CORE], BF16,
                            name=f"a2a_in{i}", uniquify=False)
                  for i in range(2)]
        a2a_out = [dram.tile([N_CORES, 128, ROWS_PER_CORE], BF16,
                             name=f"a2a_out{i}", uniquify=False)
                   for i in range(2)]

        probs_pool = top.enter_context(tc.tile_pool(name="probs", bufs=3))
        nrm_pool = top.enter_context(tc.tile_pool(name="nrm", bufs=1))

        def attn_group(p, t, psc_pool, po_pool):
            """Attention for head pair p (heads 2p, 2p+1), q-tile t."""
            qp = qpairs[p]
            nb = 4 * t + 4
            pos = [po_pool.tile([128, 512], F32, name=f"po{p}{t}{j}",
                                tag=f"po{j}") for j in range(2)]
            for b in range(nb):
                j = max(0, b - 4 * t)
                col0 = 128 * j
                diag = b >= 4 * t
                kb = kdup[:, 128 * b:128 * (b + 1)]
                qcols = (512 * t + col0, 512 * (t + 1))
                # one 2-bank psum tile holds both heads' scores side by side
                psc = psc_pool.tile([128, 1024], F32, name=f"ps{p}{t}{b}",
                                    tag="psc")
                for h in range(2):
                    nc.tensor.matmul(
                        psc[:, 512 * h + col0:512 * (h + 1)],
                        kb[64 * h:64 * (h + 1), :],
                        qp[64 * h:64 * (h + 1), qcols[0]:qcols[1]],
                        start=True, stop=True)
                probs = probs_pool.tile([128, 1024], BF16,
                                        name=f"pr{p}{t}{b}", tag="probs")
                if col0 == 0:
                    nc.scalar.activation(probs[:], psc[:], AF.Exp, scale=0.125)
                else:
                    for h in range(2):
                        nc.scalar.activation(
                            probs[:, 512 * h + col0:512 * (h + 1)],
                            psc[:, 512 * h + col0:512 * (h + 1)], AF.Exp,
                            scale=0.125)
                if diag:
                    for h in range(2):
                        nc.vector.tensor_mul(
                            probs[:, 512 * h + col0:512 * h + col0 + 128],
                            probs[:, 512 * h + col0:512 * h + col0 + 128],
                            maskT_sb[:])
                for h in range(2):
                    nc.tensor.matmul(pos[h][:, col0:512], v2sb[:, b, :],
                                     probs[:, 512 * h + col0:512 * (h + 1)],
                                     start=(b == 0), stop=(b == nb - 1))
            for h in range(2):
                # custom-DVE ops mis-handle a nonzero input base partition, so
                # stage the denominator rows at partition 0 first
                den = nrm_pool.tile([64, 512], F32, name=f"dn{p}{t}{h}",
                                    tag="den")
                nc.vector.tensor_scalar_add(den[:], pos[h][64:128, :], 0.0)
                recip = nrm_pool.tile([64, 512], F32, name=f"rc{p}{t}{h}",
                                      tag="recip")
                nc.vector.reciprocal_approx_fast(recip[:], den[:])
                nc.vector.tensor_mul(
                    attnTs[p][64 * h:64 * (h + 1), 512 * t:512 * (t + 1)],
                    pos[h][0:64, :], recip[:])

        # ---------------- Stage P: projections + rope (+ early attn) -------
        with ExitStack() as ctx:
            wpool = ctx.enter_context(tc.tile_pool(name="wpool", bufs=1))
            wq_sb = wpool.tile([128, 16, QCOLS], BF16, name="wq_sb")
            wkv_sb = wpool.tile([128, 16, KVCOLS], BF16, name="wkv_sb")
            ropeM_sb = wpool.tile([128, 4, 1024], BF16, name="ropeM_sb")
            ropeK_sb = wpool.tile([64, 4, 1024], BF16, name="ropeK_sb")
            vsb = wpool.tile([64, S], F32, name="vsb")
            identity = wpool.tile([64, 64], F32, name="identity")
            make_identity(nc, identity[:])

            nc.gpsimd.dma_start(wkv_sb[:], wkv_d.ap())
            nc.gpsimd.dma_start(ropeM_sb[:], ropeM_d.ap())
            nc.gpsimd.dma_start(ropeK_sb[:], ropeK_d.ap())
            nc.vector.memset(v2sb[:, :, HD:], 1.0)

            xtb_pool = ctx.enter_context(tc.tile_pool(name="xtb", bufs=16))
            proj_pool = ctx.enter_context(
                tc.tile_pool(name="proj", bufs=1, space="PSUM"))
            pvt_pool = ctx.enter_context(
                tc.tile_pool(name="pvt", bufs=1, space="PSUM"))
            pscE_pool = ctx.enter_context(
                tc.tile_pool(name="pscE", bufs=1, space="PSUM"))
            poE_pool = ctx.enter_context(
                tc.tile_pool(name="poE", bufs=1, space="PSUM"))
            tmp_pool = ctx.enter_context(tc.tile_pool(name="ropetmp", bufs=1))

            xtbs = {}
            for sq in range(4):
                s0 = 512 * sq
                sh, so = sq // 2, 512 * (sq % 2)
                pq = [proj_pool.tile([128, 512], F32, name=f"pq{sq}_{p}",
                                     tag=f"pq{p}") for p in range(2)]
                pkv = proj_pool.tile([128, 512], F32, name=f"pkv{sq}",
                                     tag="pkv")
                for kc in range(16):
                    if sq == 0:
                        weng = nc.scalar if kc % 2 == 0 else nc.sync
                        weng.dma_start(wq_sb[:, kc, :], wq_d.ap()[:, kc, :])
                    if sq % 2 == 0:
                        xtb = xtb_pool.tile([128, 1024], BF16,
                                            name=f"xtb{sh}_{kc}", tag="xtb")
                        eng = nc.sync if kc % 2 == 0 else nc.scalar
                        eng.dma_start(xtb[:], xT_d.ap()[sh, kc])
                        xtbs[(sh, kc)] = xtb
                    xtb = xtbs[(sh, kc)]
                    st, sp = (kc == 0), (kc == 15)
                    for p in range(2):
                        nc.tensor.matmul(
                            pq[p][:], wq_sb[:, kc, 128 * p:128 * (p + 1)],
                            xtb[:, so:so + 512], start=st, stop=sp)
                    nc.tensor.matmul(pkv[:], wkv_sb[:, kc, :],
                                     xtb[:, so:so + 512], start=st, stop=sp)
                # rope q: per pair, 4 half-muls + 4 adds (lo/hi split keeps
                # every tensor_tensor's SBUF inputs at one start partition)
                for p in range(2):
                    ta = tmp_pool.tile([64, 1024], F32, name=f"ta{sq}{p}",
                                       tag="ta")
                    tb = tmp_pool.tile([64, 1024], F32, name=f"tb{sq}{p}",
                                       tag="tb")
                    nc.vector.tensor_mul(ta[:, 0:512], pq[p][0:64, :],
                                         ropeM_sb[0:64, sq, 0:512])
                    nc.vector.tensor_mul(ta[:, 512:1024], pq[p][0:64, :],
                                         ropeM_sb[0:64, sq, 512:1024])
                    nc.vector.tensor_mul(tb[:, 0:512], pq[p][64:128, :],
                                         ropeM_sb[64:128, sq, 0:512])
                    nc.vector.tensor_mul(tb[:, 512:1024], pq[p][64:128, :],
                                         ropeM_sb[64:128, sq, 512:1024])
                    qp = qpairs[p]
                    for h in range(2):
                        a0 = 32 * h
                        nc.vector.tensor_add(
                            qp[64 * h:64 * h + 32, s0:s0 + 512],
                            ta[a0:a0 + 32, 0:512], tb[a0:a0 + 32, 0:512])
                        nc.vector.tensor_add(
                            qp[64 * h + 32:64 * h + 64, s0:s0 + 512],
                            ta[a0:a0 + 32, 512:1024],
                            tb[a0:a0 + 32, 512:1024])
                # rope k (rows 0:64 of pkv) -> kdup rows 0:64, dma-dup to 64:128
                ka = tmp_pool.tile([32, 1024], F32, name=f"ka{sq}", tag="ka")
                kb = tmp_pool.tile([32, 1024], F32, name=f"kb{sq}", tag="kb")
                nc.vector.tensor_mul(ka[:, 0:512], pkv[0:32, :],
                                     ropeK_sb[0:32, sq, 0:512])
                nc.vector.tensor_mul(ka[:, 512:1024], pkv[0:32, :],
                                     ropeK_sb[0:32, sq, 512:1024])
                nc.vector.tensor_mul(kb[:, 0:512], pkv[32:64, :],
                                     ropeK_sb[32:64, sq, 0:512])
                nc.vector.tensor_mul(kb[:, 512:1024], pkv[32:64, :],
                                     ropeK_sb[32:64, sq, 512:1024])
                nc.vector.tensor_add(kdup[0:32, s0:s0 + 512],
                                     ka[:, 0:512], kb[:, 0:512])
                nc.vector.tensor_add(kdup[32:64, s0:s0 + 512],
                                     ka[:, 512:1024], kb[:, 512:1024])
                nc.sync.dma_start(kdup[64:128, s0:s0 + 512],
                                  kdup[0:64, s0:s0 + 512])
                # v -> vsb (f32) -> per-128-block transpose -> v2sb cols 0:64
                nc.scalar.copy(vsb[:, s0:s0 + 512], pkv[64:128, :])
                for sc in range(4 * sq, 4 * sq + 4):
                    pvt = pvt_pool.tile([128, 64], F32, name=f"pvt{sc}",
                                        tag="pvt")
                    nc.tensor.transpose(pvt[:], vsb[:, 128 * sc:128 * (sc + 1)],
                                        identity[:])
                    nc.scalar.copy(v2sb[:, sc, 0:HD], pvt[:])
                if sq >= 1:
                    attn_group(0, sq - 1, pscE_pool, poE_pool)
                if sq == 3:
                    attn_group(0, 3, pscE_pool, poE_pool)


        # ---------------- Phase 2: rest of attention + A2A + stage W --------
        with ExitStack() as ctx:
            pscL_pool = ctx.enter_context(
                tc.tile_pool(name="pscL", bufs=2, space="PSUM"))
            poL_pool = ctx.enter_context(
                tc.tile_pool(name="poL", bufs=1, space="PSUM"))
            pw_pool = ctx.enter_context(
                tc.tile_pool(name="pw", bufs=1, space="PSUM"))
            af_pool = ctx.enter_context(tc.tile_pool(name="af", bufs=1))
            osb_pool = ctx.enter_context(tc.tile_pool(name="osb", bufs=2))
            accp = ctx.enter_context(tc.tile_pool(name="accp", bufs=1))
            acc_sb = accp.tile([128, 8, 512], F32, name="acc_sb")

            def send_a2a(i):
                for r in range(N_CORES):
                    nc.sync.dma_start(a2a_in[i][r],
                                      attnTs[i][:, 256 * r:256 * (r + 1)])
                nc.gpsimd.collective_compute(
                    "AllToAll", mybir.AluOpType.bypass,
                    replica_groups=[list(range(N_CORES))],
                    ins=[a2a_in[i][:]], outs=[a2a_out[i][:]])

            afs = []

            def load_af(i):
                af = af_pool.tile([128, N_CORES, ROWS_PER_CORE], BF16,
                                  name=f"attn_full{i}", uniquify=False)
                nc.sync.dma_start(af[:],
                                  a2a_out[i][:].rearrange("r p s -> p r s"))
                afs.append(af)

            def w_subpass(i, m, n2):
                # accumulate out[128m:128m+128, 1024n2:1024n2+1024] over the 8
                # src cores of a2a chunk i, two psum banks (tags A/B)
                for nn in range(2):
                    n = 2 * n2 + nn
                    pw = pw_pool.tile([128, 512], F32, name=f"pw{i}{m}{n}",
                                      tag=f"pw{nn}")
                    for r in range(N_CORES):
                        nc.tensor.matmul(
                            pw[:], afs[i][:, r, 128 * m:128 * (m + 1)],
                            wo_sb[:, 2 * r + i, 512 * n:512 * (n + 1)],
                            start=(r == 0), stop=(r == N_CORES - 1))
                    if i == 0:
                        nc.vector.tensor_scalar_add(acc_sb[:, 4 * m + n, :],
                                                    pw[:], 0.0)
                    else:
                        osb = osb_pool.tile([128, 512], F32,
                                            name=f"osb{m}{n}", tag="osb")
                        nc.vector.tensor_add(osb[:], pw[:],
                                             acc_sb[:, 4 * m + n, :])
                        nc.sync.dma_start(
                            out_d.ap()[128 * m:128 * (m + 1),
                                       512 * n:512 * (n + 1)], osb[:])

            send_a2a(0)
            load_af(0)
            for kc in range(16):
                nc.gpsimd.tensor_scalar_add(wo_sb[:, kc, 0:1],
                                            kdup[:, 0:1], 0.0)
                nc.gpsimd.dma_start(wo_sb[:, kc, :], wo_d.ap()[:, kc, :])
            attn_group(1, 0, pscL_pool, poL_pool)
            attn_group(1, 1, pscL_pool, poL_pool)
            attn_group(1, 2, pscL_pool, poL_pool)
            attn_group(1, 3, pscL_pool, poL_pool)
            send_a2a(1)
            for m in range(2):
                for n2 in range(2):
                    w_subpass(0, m, n2)
            load_af(1)
            for m in range(2):
                for n2 in range(2):
                    w_subpass(1, m, n2)

    nc.compile()
    return nc


_NC_CACHE = None
LAST_RESULT = None


def _get_nc():
    global _NC_CACHE
    if _NC_CACHE is None:
        _NC_CACHE = _build()
    return _NC_CACHE


def kernel(x, wq, wk, wv, wo, freqs_cos, freqs_sin, mask, start_pos=0):
    assert int(start_pos) == 0, "kernel specialized for start_pos == 0"
    import ml_dtypes
    x = np.asarray(x, np.float32)
    b, s, d = x.shape
    assert (b, s, d) == (1, S, D)
    xT = np.ascontiguousarray(x[0].T).astype(ml_dtypes.bfloat16)
    # pre-tile: xT[sh, kc] = contiguous (128, 1024) block -> 2KB DMA lines
    xTt = np.ascontiguousarray(
        xT.reshape(16, 128, 2, 1024).transpose(2, 0, 1, 3))

    # wq pair-packed: per pair of heads, cols = [h0 evens, h1 evens,
    # h0 odds, h1 odds]
    wq_f = np.asarray(wq, np.float32).reshape(D, 32, 32, 2)
    wk_f = np.asarray(wk, np.float32).reshape(D, 8, 32, 2)
    wv_f = np.asarray(wv, np.float32)

    cosT = np.asarray(freqs_cos, np.float32).T  # (32, S)
    sinT = np.asarray(freqs_sin, np.float32).T
    ropeM = np.empty((128, 4, 1024), np.float32)
    ropeK = np.empty((64, 4, 1024), np.float32)
    for sq in range(4):
        c = cosT[:, 512 * sq:512 * (sq + 1)]
        sn = sinT[:, 512 * sq:512 * (sq + 1)]
        ropeM[0:32, sq, 0:512] = c
        ropeM[32:64, sq, 0:512] = c
        ropeM[64:96, sq, 0:512] = -sn
        ropeM[96:128, sq, 0:512] = -sn
        ropeM[0:32, sq, 512:] = sn
        ropeM[32:64, sq, 512:] = sn
        ropeM[64:96, sq, 512:] = c
        ropeM[96:128, sq, 512:] = c
        ropeK[0:32, sq, 0:512] = c
        ropeK[32:64, sq, 0:512] = -sn
        ropeK[0:32, sq, 512:] = sn
        ropeK[32:64, sq, 512:] = c
    ropeM_b = ropeM.astype(ml_dtypes.bfloat16)
    ropeK_b = ropeK.astype(ml_dtypes.bfloat16)

    wot = np.ascontiguousarray(
        np.asarray(wo, np.float32).reshape(16, 128, D).transpose(1, 0, 2)
    ).astype(ml_dtypes.bfloat16)
    maskT01 = np.ascontiguousarray(
        (np.asarray(mask, np.float32)[:128, :128].T == 0.0)
    ).astype(ml_dtypes.bfloat16)

    in_maps = []
    for c in range(N_CORES):
        # pair p cols: h=4c+2p, h2=4c+2p+1
        wq_cols = []
        for p in range(2):
            h0, h1 = 4 * c + 2 * p, 4 * c + 2 * p + 1
            wq_cols.append(wq_f[:, h0, :, 0])  # evens (D, 32)
            wq_cols.append(wq_f[:, h1, :, 0])
            wq_cols.append(wq_f[:, h0, :, 1])  # odds
            wq_cols.append(wq_f[:, h1, :, 1])
        wq_c = np.concatenate(wq_cols, axis=1)  # (D, 256)
        wkv_c = np.concatenate(
            [wk_f[:, c, :, 0], wk_f[:, c, :, 1],
             wv_f[:, HD * c:HD * (c + 1)]], axis=1)  # (D, 128)
        in_maps.append({
            "xT": xTt,
            "wq": np.ascontiguousarray(
                wq_c.reshape(16, 128, QCOLS).transpose(1, 0, 2)
            ).astype(ml_dtypes.bfloat16),
            "wkv": np.ascontiguousarray(
                wkv_c.reshape(16, 128, KVCOLS).transpose(1, 0, 2)
            ).astype(ml_dtypes.bfloat16),
            "wo": wot,
            "ropeM": ropeM_b,
            "ropeK": ropeK_b,
            "maskT01": maskT01,
        })

    nc = _get_nc()
    res = bass_utils.run_bass_kernel_spmd(
        nc, in_maps, core_ids=list(range(N_CORES)),
        trace=bool(os.environ.get("BASS_TRACE")))
    global LAST_RESULT
    LAST_RESULT = res
    rows = [res.results[c]["out"] for c in range(N_CORES)]
    return np.concatenate(rows, axis=0).reshape(1, S, D).astype(np.float32)


# revision 15
# speedup vs baseline: 1.1239x; 1.0525x over previous
"""GQA attention (S=2048, D=2048, 32 q-heads / 8 kv-heads, rope, causal) on 8
Trainium2 NeuronCores, tensor-parallel over heads (1 kv head + 4 q heads per
core), chunked AllToAll re-shard overlapped with compute, row-sharded output.

v2 layout/scheduling notes (on top of the v1 transposed-domain design):
 - rope is vectorized across a whole 128-partition psum tile: q heads are
   pair-packed ([h0 evens; h1 evens; h0 odds; h1 odds] rows) so one DVE mul
   against a host-built [cos;cos;-sin;-sin | sin;sin;cos;cos] table plus four
   32-row adds replaces 24 narrow ops per tile.
 - scores for the two heads of a pair run CONCURRENTLY on the PE via row
   tiling: kT is duplicated to partitions 64-127, q pairs live at [0:64] and
   [64:128], so the two K=64 matmuls occupy disjoint row groups.
 - softmax denominators: the PV stationary is [v | ones*64] so psum rows
   64-127 hold 64 replicated copies of the denominator -> reciprocal+scale are
   plain [64,512] DVE ops (no gpsimd partition_broadcast chain).
 - stage P (projections) and early attention tiles of head-pair 0 are
   interleaved so the scalar engine starts exp'ing while projections stream.
 - stage W runs in 2 psum banks with an SBUF accumulator so its first
   AllToAll chunk overlaps head-pair-1 attention; second chunk is the tail.
"""
import os
import sys
from contextlib import ExitStack

import numpy as np

try:
    import concourse.bass as bass  # noqa: F401
except ImportError:  # platform tree not on sys.path in a fresh dir
    sys.path.insert(0, "/opt/trn_rl_repo")
    import concourse.bass as bass  # noqa: F401

import concourse.mybir as mybir
from concourse import bacc, bass_utils, tile
from concourse.masks import make_identity

F32 = mybir.dt.float32
BF16 = mybir.dt.bfloat16
AF = mybir.ActivationFunctionType

S = 2048          # sequence length
D = 2048          # model dim
HD = 64           # head dim
N_CORES = 8
QCOLS = 256       # 4 q heads * 64 per core (2 pairs of 128)
KVCOLS = 128      # packed k(evens,odds)|v cols per core
ROWS_PER_CORE = S // N_CORES  # 256 output rows per core

# fraction of non-diagonal exp blocks computed on the vector engine via
# pow(e^0.125, x); 0 = all on scalar engine
DVE_EXP = False


def _build():
    nc = bacc.Bacc("TRN2", target_bir_lowering=False, debug=False,
                   num_devices=N_CORES)
    xT_d = nc.dram_tensor("xT", [2, 16, 128, 1024], BF16,
                          kind="ExternalInput")
    wq_d = nc.dram_tensor("wq", [128, 16, QCOLS], BF16, kind="ExternalInput")
    wkv_d = nc.dram_tensor("wkv", [128, 16, KVCOLS], BF16,
                           kind="ExternalInput")
    wo_d = nc.dram_tensor("wo", [128, 16, D], BF16, kind="ExternalInput")
    ropeM_d = nc.dram_tensor("ropeM", [128, 4, 1024], BF16,
                             kind="ExternalInput")
    ropeK_d = nc.dram_tensor("ropeK", [64, 4, 1024], BF16,
                             kind="ExternalInput")
    mask_d = nc.dram_tensor("maskT01", [128, 128], BF16, kind="ExternalInput")
    out_d = nc.dram_tensor("out", [ROWS_PER_CORE, D], F32,
                           kind="ExternalOutput")

    with tile.TileContext(nc) as tc, ExitStack() as top:
        persist = top.enter_context(tc.tile_pool(name="persist", bufs=1))
        qpairs = [persist.tile([128, S], BF16, name=f"qpair{p}",
                               uniquify=False) for p in range(2)]
        kdup = persist.tile([128, S], BF16, name="kdup")
        v2sb = persist.tile([128, 16, 128], BF16, name="v2sb")
        attnT0 = persist.tile([128, S], BF16, name="attnT0")
        attnT1 = persist.tile([128, S], BF16, name="attnT1")
        attnTs = [attnT0, attnT1]
        maskT_sb = persist.tile([128, 128], BF16, name="maskT_sb")
        nc.scalar.dma_start(maskT_sb[:], mask_d.ap())
        wo_sb = persist.tile([128, 16, D], BF16, name="wo_sb")
        if DVE_EXP:
            expbase = persist.tile([128, 512], F32, name="expbase")
            nc.vector.memset(expbase[:], float(np.exp(0.125)))

        dram = top.enter_context(tc.tile_pool(name="dram", bufs=1,
                                              space="DRAM"))
        a2a_in = [dram.tile([N_CORES, 128, ROWS_PER_CORE], BF16,
                            name=f"a2a_in{i}", uniquify=False)
                  for i in range(2)]
        a2a_out = [dram.tile([N_CORES, 128, ROWS_PER_CORE], BF16,
                             name=f"a2a_out{i}", uniquify=False)
                   for i in range(2)]

        probs_pool = top.enter_context(tc.tile_pool(name="probs", bufs=3))
        nrm_pool = top.enter_context(tc.tile_pool(name="nrm", bufs=1))

        def attn_group(p, t, psc_pool, po_pool):
            """Attention for head pair p (heads 2p, 2p+1), q-tile t."""
            qp = qpairs[p]
            nb = 4 * t + 4
            pos = [po_pool.tile([128, 512], F32, name=f"po{p}{t}{j}",
                                tag=f"po{j}") for j in range(2)]
            for b in range(nb):
                j = max(0, b - 4 * t)
                col0 = 128 * j
                diag = b >= 4 * t
                kb = kdup[:, 128 * b:128 * (b + 1)]
                qcols = (512 * t + col0, 512 * (t + 1))
                # one 2-bank psum tile holds both heads' scores side by side
                psc = psc_pool.tile([128, 1024], F32, name=f"ps{p}{t}{b}",
                                    tag="psc")
                for h in range(2):
                    nc.tensor.matmul(
                        psc[:, 512 * h + col0:512 * (h + 1)],
                        kb[64 * h:64 * (h + 1), :],
                        qp[64 * h:64 * (h + 1), qcols[0]:qcols[1]],
                        start=True, stop=True)
                probs = probs_pool.tile([128, 1024], BF16,
                                        name=f"pr{p}{t}{b}", tag="probs")
                if col0 == 0:
                    nc.scalar.activation(probs[:], psc[:], AF.Exp, scale=0.125)
                else:
                    for h in range(2):
                        nc.scalar.activation(
                            probs[:, 512 * h + col0:512 * (h + 1)],
                            psc[:, 512 * h + col0:512 * (h + 1)], AF.Exp,
                            scale=0.125)
                if diag:
                    for h in range(2):
                        nc.vector.tensor_mul(
                            probs[:, 512 * h + col0:512 * h + col0 + 128],
                            probs[:, 512 * h + col0:512 * h + col0 + 128],
                            maskT_sb[:])
                for h in range(2):
                    nc.tensor.matmul(pos[h][:, col0:512], v2sb[:, b, :],
                                     probs[:, 512 * h + col0:512 * (h + 1)],
                                     start=(b == 0), stop=(b == nb - 1))
            for h in range(2):
                # custom-DVE ops mis-handle a nonzero input base partition, so
                # stage the denominator rows at partition 0 first
                den = nrm_pool.tile([64, 512], F32, name=f"dn{p}{t}{h}",
                                    tag="den")
                nc.vector.tensor_scalar_add(den[:], pos[h][64:128, :], 0.0)
                recip = nrm_pool.tile([64, 512], F32, name=f"rc{p}{t}{h}",
                                      tag="recip")
                nc.vector.reciprocal_approx_fast(recip[:], den[:])
                nc.vector.tensor_mul(
                    attnTs[p][64 * h:64 * (h + 1), 512 * t:512 * (t + 1)],
                    pos[h][0:64, :], recip[:])

        # ---------------- Stage P: projections + rope (+ early attn) -------
        with ExitStack() as ctx:
            wpool = ctx.enter_context(tc.tile_pool(name="wpool", bufs=1))
            wq_sb = wpool.tile([128, 16, QCOLS], BF16, name="wq_sb")
            wkv_sb = wpool.tile([128, 16, KVCOLS], BF16, name="wkv_sb")
            ropeM_sb = wpool.tile([128, 4, 1024], BF16, name="ropeM_sb")
            ropeK_sb = wpool.tile([64, 4, 1024], BF16, name="ropeK_sb")
            vsb = wpool.tile([64, S], F32, name="vsb")
            identity = wpool.tile([64, 64], F32, name="identity")
            make_identity(nc, identity[:])

            nc.gpsimd.dma_start(wkv_sb[:], wkv_d.ap())
            nc.gpsimd.dma_start(ropeM_sb[:], ropeM_d.ap())
            nc.gpsimd.dma_start(ropeK_sb[:], ropeK_d.ap())
            nc.vector.memset(v2sb[:, :, HD:], 1.0)

            xtb_pool = ctx.enter_context(tc.tile_pool(name="xtb", bufs=16))
            proj_pool = ctx.enter_context(
                tc.tile_pool(name="proj", bufs=1, space="PSUM"))
            pvt_pool = ctx.enter_context(
                tc.tile_pool(name="pvt", bufs=1, space="PSUM"))
            pscE_pool = ctx.enter_context(
                tc.tile_pool(name="pscE", bufs=1, space="PSUM"))
            poE_pool = ctx.enter_context(
                tc.tile_pool(name="poE", bufs=1, space="PSUM"))
            tmp_pool = ctx.enter_context(tc.tile_pool(name="ropetmp", bufs=1))

            xtbs = {}
            for sq in range(4):
                s0 = 512 * sq
                sh, so = sq // 2, 512 * (sq % 2)
                pq = [proj_pool.tile([128, 512], F32, name=f"pq{sq}_{p}",
                                     tag=f"pq{p}") for p in range(2)]
                pkv = proj_pool.tile([128, 512], F32, name=f"pkv{sq}",
                                     tag="pkv")
                for kc in range(16):
                    if sq == 0:
                        weng = nc.scalar if kc % 2 == 0 else nc.sync
                        weng.dma_start(wq_sb[:, kc, :], wq_d.ap()[:, kc, :])
                    if sq % 2 == 0:
                        xtb = xtb_pool.tile([128, 1024], BF16,
                                            name=f"xtb{sh}_{kc}", tag="xtb")
                        eng = nc.sync if kc % 2 == 0 else nc.scalar
                        eng.dma_start(xtb[:], xT_d.ap()[sh, kc])
                        xtbs[(sh, kc)] = xtb
                    xtb = xtbs[(sh, kc)]
                    st, sp = (kc == 0), (kc == 15)
                    for p in range(2):
                        nc.tensor.matmul(
                            pq[p][:], wq_sb[:, kc, 128 * p:128 * (p + 1)],
                            xtb[:, so:so + 512], start=st, stop=sp)
                    nc.tensor.matmul(pkv[:], wkv_sb[:, kc, :],
                                     xtb[:, so:so + 512], start=st, stop=sp)
                # rope q: per pair, 4 half-muls + 4 adds (lo/hi split keeps
                # every tensor_tensor's SBUF inputs at one start partition)
                for p in range(2):
                    ta = tmp_pool.tile([64, 1024], F32, name=f"ta{sq}{p}",
                                       tag="ta")
                    tb = tmp_pool.tile([64, 1024], F32, name=f"tb{sq}{p}",
                                       tag="tb")
                    nc.vector.tensor_mul(ta[:, 0:512], pq[p][0:64, :],
                                         ropeM_sb[0:64, sq, 0:512])
                    nc.vector.tensor_mul(ta[:, 512:1024], pq[p][0:64, :],
                                         ropeM_sb[0:64, sq, 512:1024])
                    nc.vector.tensor_mul(tb[:, 0:512], pq[p][64:128, :],
                                         ropeM_sb[64:128, sq, 0:512])
                    nc.vector.tensor_mul(tb[:, 512:1024], pq[p][64:128, :],
                                         ropeM_sb[64:128, sq, 512:1024])
                    qp = qpairs[p]
                    for h in range(2):
                        a0 = 32 * h
                        nc.vector.tensor_add(
                            qp[64 * h:64 * h + 32, s0:s0 + 512],
                            ta[a0:a0 + 32, 0:512], tb[a0:a0 + 32, 0:512])
                        nc.vector.tensor_add(
                            qp[64 * h + 32:64 * h + 64, s0:s0 + 512],
                            ta[a0:a0 + 32, 512:1024],
                            tb[a0:a0 + 32, 512:1024])
                # rope k (rows 0:64 of pkv) -> kdup rows 0:64, dma-dup to 64:128
                ka = tmp_pool.tile([32, 1024], F32, name=f"ka{sq}", tag="ka")
                kb = tmp_pool.tile([32, 1024], F32, name=f"kb{sq}", tag="kb")
                nc.vector.tensor_mul(ka[:, 0:512], pkv[0:32, :],
                                     ropeK_sb[0:32, sq, 0:512])
                nc.vector.tensor_mul(ka[:, 512:1024], pkv[0:32, :],
                                     ropeK_sb[0:32, sq, 512:1024])
                nc.vector.tensor_mul(kb[:, 0:512], pkv[32:64, :],
                                     ropeK_sb[32:64, sq, 0:512])
                nc.vector.tensor_mul(kb[:, 512:1024], pkv[32:64, :],
                                     ropeK_sb[32:64, sq, 512:1024])
                nc.vector.tensor_add(kdup[0:32, s0:s0 + 512],
                                     ka[:, 0:512], kb[:, 0:512])
                nc.vector.tensor_add(kdup[32:64, s0:s0 + 512],
                                     ka[:, 512:1024], kb[:, 512:1024])
                nc.sync.dma_start(kdup[64:128, s0:s0 + 512],
                                  kdup[0:64, s0:s0 + 512])
                # v -> vsb (f32) -> per-128-block transpose -> v2sb cols 0:64
                nc.scalar.copy(vsb[:, s0:s0 + 512], pkv[64:128, :])
                for sc in range(4 * sq, 4 * sq + 4):
                    pvt = pvt_pool.tile([128, 64], F32, name=f"pvt{sc}",
                                        tag="pvt")
                    nc.tensor.transpose(pvt[:], vsb[:, 128 * sc:128 * (sc + 1)],
                                        identity[:])
                    nc.scalar.copy(v2sb[:, sc, 0:HD], pvt[:])
                if sq >= 1:
                    attn_group(1, sq - 1, pscE_pool, poE_pool)
                if sq == 3:
                    attn_group(1, 3, pscE_pool, poE_pool)


        # ---------------- Phase 2: rest of attention + A2A + stage W --------
        with ExitStack() as ctx:
            pscL_pool = ctx.enter_context(
                tc.tile_pool(name="pscL", bufs=2, space="PSUM"))
            poL_pool = ctx.enter_context(
                tc.tile_pool(name="poL", bufs=1, space="PSUM"))
            pw_pool = ctx.enter_context(
                tc.tile_pool(name="pw", bufs=1, space="PSUM"))
            af_pool = ctx.enter_context(tc.tile_pool(name="af", bufs=1))
            osb_pool = ctx.enter_context(tc.tile_pool(name="osb", bufs=2))
            accp = ctx.enter_context(tc.tile_pool(name="accp", bufs=1))
            acc_sb = accp.tile([128, 8, 512], F32, name="acc_sb")

            def send_a2a(i):
                for r in range(N_CORES):
                    nc.sync.dma_start(a2a_in[i][r],
                                      attnTs[i][:, 256 * r:256 * (r + 1)])
                nc.gpsimd.collective_compute(
                    "AllToAll", mybir.AluOpType.bypass,
                    replica_groups=[list(range(N_CORES))],
                    ins=[a2a_in[i][:]], outs=[a2a_out[i][:]])

            afs = {}

            def load_af(i):
                af = af_pool.tile([128, N_CORES, ROWS_PER_CORE], BF16,
                                  name=f"attn_full{i}", uniquify=False)
                nc.sync.dma_start(af[:],
                                  a2a_out[i][:].rearrange("r p s -> p r s"))
                afs[i] = af

            def w_subpass(i, m, n2, first):
                # accumulate out[128m:128m+128, 1024n2:1024n2+1024] over the 8
                # src cores of a2a chunk i, two psum banks (tags A/B)
                for nn in range(2):
                    n = 2 * n2 + nn
                    pw = pw_pool.tile([128, 512], F32, name=f"pw{i}{m}{n}",
                                      tag=f"pw{nn}")
                    for r in range(N_CORES):
                        nc.tensor.matmul(
                            pw[:], afs[i][:, r, 128 * m:128 * (m + 1)],
                            wo_sb[:, 2 * r + i, 512 * n:512 * (n + 1)],
                            start=(r == 0), stop=(r == N_CORES - 1))
                    if first:
                        nc.vector.tensor_scalar_add(acc_sb[:, 4 * m + n, :],
                                                    pw[:], 0.0)
                    else:
                        osb = osb_pool.tile([128, 512], F32,
                                            name=f"osb{m}{n}", tag="osb")
                        nc.vector.tensor_add(osb[:], pw[:],
                                             acc_sb[:, 4 * m + n, :])
                        nc.sync.dma_start(
                            out_d.ap()[128 * m:128 * (m + 1),
                                       512 * n:512 * (n + 1)], osb[:])

            send_a2a(1)
            load_af(1)
            for kc in range(16):
                nc.gpsimd.tensor_scalar_add(wo_sb[:, kc, 0:1],
                                            kdup[:, 0:1], 0.0)
                nc.gpsimd.dma_start(wo_sb[:, kc, :], wo_d.ap()[:, kc, :])
            attn_group(0, 0, pscL_pool, poL_pool)
            attn_group(0, 1, pscL_pool, poL_pool)
            attn_group(0, 2, pscL_pool, poL_pool)
            attn_group(0, 3, pscL_pool, poL_pool)
            send_a2a(0)
            for m in range(2):
                for n2 in range(2):
                    w_subpass(1, m, n2, first=True)
            load_af(0)
            for m in range(2):
                for n2 in range(2):
                    w_subpass(0, m, n2, first=False)

    nc.compile()
    return nc


_NC_CACHE = None
LAST_RESULT = None


def _get_nc():
    global _NC_CACHE
    if _NC_CACHE is None:
        _NC_CACHE = _build()
    return _NC_CACHE


def kernel(x, wq, wk, wv, wo, freqs_cos, freqs_sin, mask, start_pos=0):
    assert int(start_pos) == 0, "kernel specialized for start_pos == 0"
    import ml_dtypes
    x = np.asarray(x, np.float32)
    b, s, d = x.shape
    assert (b, s, d) == (1, S, D)
    xT = np.ascontiguousarray(x[0].T).astype(ml_dtypes.bfloat16)
    # pre-tile: xT[sh, kc] = contiguous (128, 1024) block -> 2KB DMA lines
    xTt = np.ascontiguousarray(
        xT.reshape(16, 128, 2, 1024).transpose(2, 0, 1, 3))

    # wq pair-packed: per pair of heads, cols = [h0 evens, h1 evens,
    # h0 odds, h1 odds]
    wq_f = np.asarray(wq, np.float32).reshape(D, 32, 32, 2)
    wk_f = np.asarray(wk, np.float32).reshape(D, 8, 32, 2)
    wv_f = np.asarray(wv, np.float32)

    cosT = np.asarray(freqs_cos, np.float32).T  # (32, S)
    sinT = np.asarray(freqs_sin, np.float32).T
    ropeM = np.empty((128, 4, 1024), np.float32)
    ropeK = np.empty((64, 4, 1024), np.float32)
    for sq in range(4):
        c = cosT[:, 512 * sq:512 * (sq + 1)]
        sn = sinT[:, 512 * sq:512 * (sq + 1)]
        ropeM[0:32, sq, 0:512] = c
        ropeM[32:64, sq, 0:512] = c
        ropeM[64:96, sq, 0:512] = -sn
        ropeM[96:128, sq, 0:512] = -sn
        ropeM[0:32, sq, 512:] = sn
        ropeM[32:64, sq, 512:] = sn
        ropeM[64:96, sq, 512:] = c
        ropeM[96:128, sq, 512:] = c
        ropeK[0:32, sq, 0:512] = c
        ropeK[32:64, sq, 0:512] = -sn
        ropeK[0:32, sq, 512:] = sn
        ropeK[32:64, sq, 512:] = c
    ropeM_b = ropeM.astype(ml_dtypes.bfloat16)
    ropeK_b = ropeK.astype(ml_dtypes.bfloat16)

    wot = np.ascontiguousarray(
        np.asarray(wo, np.float32).reshape(16, 128, D).transpose(1, 0, 2)
    ).astype(ml_dtypes.bfloat16)
    maskT01 = np.ascontiguousarray(
        (np.asarray(mask, np.float32)[:128, :128].T == 0.0)
    ).astype(ml_dtypes.bfloat16)

    in_maps = []
    for c in range(N_CORES):
        # pair p cols: h=4c+2p, h2=4c+2p+1
        wq_cols = []
        for p in range(2):
            h0, h1 = 4 * c + 2 * p, 4 * c + 2 * p + 1
            wq_cols.append(wq_f[:, h0, :, 0])  # evens (D, 32)
            wq_cols.append(wq_f[:, h1, :, 0])
            wq_cols.append(wq_f[:, h0, :, 1])  # odds
            wq_cols.append(wq_f[:, h1, :, 1])
        wq_c = np.concatenate(wq_cols, axis=1)  # (D, 256)
        wkv_c = np.concatenate(
            [wk_f[:, c, :, 0], wk_f[:, c, :, 1],
             wv_f[:, HD * c:HD * (c + 1)]], axis=1)  # (D, 128)
        in_maps.append({
            "xT": xTt,
            "wq": np.ascontiguousarray(
                wq_c.reshape(16, 128, QCOLS).transpose(1, 0, 2)
            ).astype(ml_dtypes.bfloat16),
            "wkv": np.ascontiguousarray(
                wkv_c.reshape(16, 128, KVCOLS).transpose(1, 0, 2)
            ).astype(ml_dtypes.bfloat16),
            "wo": wot,
            "ropeM": ropeM_b,
            "ropeK": ropeK_b,
            "maskT01": maskT01,
        })

    nc = _get_nc()
    res = bass_utils.run_bass_kernel_spmd(
        nc, in_maps, core_ids=list(range(N_CORES)),
        trace=bool(os.environ.get("BASS_TRACE")))
    global LAST_RESULT
    LAST_RESULT = res
    rows = [res.results[c]["out"] for c in range(N_CORES)]
    return np.concatenate(rows, axis=0).reshape(1, S, D).astype(np.float32)
